# revision 55
# baseline (speedup 1.0000x reference)
"""Distributed (8-NeuronCore SPMD) Trainium2 Bass kernel: masked multi-head attention.

Problem: x[4,2048,1024] -> qkv (16 heads, d=64) -> masked softmax attention -> proj.

Sharding (Megatron-style, per sharding hint):
  core c -> batch b = c//2, head-group g = c%2 (8 heads per core).
  Wqkv columns / Wproj rows are split by head group; each core computes a full
  [2048,1024] *partial* projection output for its batch; the host sums the two
  tensor-parallel partials per batch (bproj is fed to group 0 only).

Per-core layout (PSUM-bank-economy design):
  - Loop nest: head-pair hp OUTER (4), q-block qb INNER (4 x 512 tokens),
    m-chunk mi INNERMOST (16 x 128 tokens).
  - Scores S^T[m, q] per head pair land in one [128, 2, 512] f32 PSUM tile
    (2 banks); two such tiles double-buffer (4 banks).  The two heads' score
    matmuls go to disjoint PE row groups (partitions 0/64) and different
    banks, so they issue back-to-back and run concurrently on the array.
  - One merged ACT exp per mi covers both heads ([128, 1024] -> bf16 pt);
    ScalarE paces the loop at ~1.15us per m-chunk.
  - Keep-mask is bf16 (so the DVE multiply runs in 2x packed mode) and is
    streamed from HBM per unit ([128, 16, 512] tiles, double-buffered,
    prefetched during the previous unit; re-read once per head-pair pass).
  - PV: per head a [65, 512] f32 PSUM accumulator (1 bank each; V carries a
    ones column so row 64 accumulates the softmax denominator; normalization
    reads the accumulator directly from PSUM).
  - The remaining 2 PSUM banks host ALL filler matmuls as 1-bank [*, 512]
    tiles: qkv slices for hp+1, V chunks (inside the first unit), the
    denominator broadcast, and the output projection.  Filler work is split
    into single-matmul quanta drained a few per m-chunk, because Tile's
    cross-engine waits count the producer engine's stream position: anything
    scheduled ahead of a score matmul delays the exp that depends on it.
  - proj(qb) is emitted as soon as norm(hp=3, qb) is available, overlapping
    the last head-pair pass; output streamed per [128, 512] tile.
  - A dummy matmul burst at kernel start flips the PE HAM clock-gate to 8/8
    while x loads, so qkv(0) runs at 2.4 GHz.
"""

import os

import numpy as np
import ml_dtypes

BF16 = ml_dtypes.bfloat16

B, N, DIM, HEADS = 4, 2048, 1024, 16
HL = HEADS // 2        # heads per core = 8
D = DIM // HEADS       # head dim = 64
CL = DIM // 2          # local channels per core = 512
P = 128
NCH = N // P           # 16 token chunks
CCH = DIM // P         # 8 contraction chunks
VW = D + 1             # 65: V plus ones column
QB = 512               # q-block width
NQB = N // QB          # 4 q-blocks

_nc_cache = None
LAST_EXEC_NS = None
LAST_RESULTS = None


def _body(tc, nc, mybir, xT, wqk, wv, bqk, bv, maskk, wp, bp, out):
    import concourse.bass as bass  # noqa: F401

    f32 = mybir.dt.float32
    bf16 = mybir.dt.bfloat16
    Exp = mybir.ActivationFunctionType.Exp
    HP = HL // 2  # head pairs per core = 4

    with (
        tc.tile_pool(name="persist", bufs=1) as pers,
        tc.tile_pool(name="wqkp", bufs=2) as wqkpool,
        tc.tile_pool(name="pp", bufs=12) as ppool,     # pt (masked exp) tiles
        tc.tile_pool(name="mq", bufs=2) as mqpool,     # streamed bf16 keep-mask
        tc.tile_pool(name="op", bufs=4) as opool,      # ost / staging f32
        tc.tile_pool(name="sp", bufs=4) as spool,      # small norm tiles
        tc.tile_pool(name="zp", bufs=3) as zpool,      # proj staging
        tc.tile_pool(name="st", bufs=2, space="PSUM") as stp,    # 4 banks
        tc.tile_pool(name="ot", bufs=2, space="PSUM") as otp_p,  # 2 banks
        tc.tile_pool(name="ln", bufs=2, space="PSUM") as lend,   # 2 banks
    ):
        # ---------------- persistent tiles ----------------
        xt = pers.tile([P, CCH, N], bf16, name="xt")          # x^T, [c, n]
        vsb = pers.tile([P, NCH, HL * VW], bf16, name="vsb")  # V natural, [V | 1] per head
        wpsb = pers.tile([P, CL // P, DIM], bf16, name="wpsb")
        onrm = pers.tile([P, CL // P, N], bf16, name="onrm")  # normalized O^T, [c, n]
        qksb = pers.tile([P, HP, 2, N], bf16, name="qksb")    # Q^T,K^T all head pairs
        wvsb = pers.tile([P, CCH, CL], bf16, name="wvsb")
        bqksb = pers.tile([P, 2 * CL // P], f32, name="bqksb")
        bvsb = pers.tile([1, CL], bf16, name="bvsb")
        bpsb = pers.tile([1, DIM], bf16, name="bpsb")
        ones = pers.tile([1, P], bf16, name="ones")
        ones5 = pers.tile([1, QB], bf16, name="ones5")

        nc.vector.memset(ones[:], 1.0)
        nc.vector.memset(ones5[:], 1.0)
        nc.vector.memset(
            vsb.rearrange("p t (h c) -> p t h c", c=VW)[:, :, :, D], 1.0
        )
        # x first (everything depends on it), one big transfer per queue
        for c in range(CCH):
            nc.sync.dma_start(xt[:, c, :], xT[c * P:(c + 1) * P, :])
        # dense dummy matmul burst: flips the PE HAM clock-gate to 8/8
        # during the x DMA so qkv(0) runs at 2.4 GHz, not 1.2
        warm = lend.tile([P, QB], f32, name="warm", tag="ln")
        for i in range(20):
            nc.tensor.matmul(warm[:, :], ones[:, :], ones5[:, :],
                             start=(i == 0), stop=(i == 19))
        nc.sync.dma_start(bqksb[:], bqk.rearrange("(o p) -> p o", p=P))
        nc.sync.dma_start(bvsb[:], bv[:])

        def load_wq(hp):
            wq = wqkpool.tile([P, CCH, 2 * P], bf16, name=f"wq{hp}", tag="wqk")
            for c in range(CCH):
                nc.sync.dma_start(
                    wq[:, c, :P], wqk[c * P:(c + 1) * P, P * hp:P * (hp + 1)]
                )
                nc.sync.dma_start(
                    wq[:, c, P:], wqk[c * P:(c + 1) * P, CL + P * hp:CL + P * (hp + 1)]
                )
            return wq

        QKV_ORDER = [(0, 0)] + [(1, t4) for t4 in range(N // QB)] + [
            (0, t4) for t4 in range(1, N // QB)
        ]

        def qkv_quanta(hp, wq):
            # One thunk per matmul so filler work interleaves at ~215ns
            # granularity inside the ACT-paced loop; the lend slot for a
            # slice is held across its 8 accumulation quanta.
            quanta = []
            for j01, t4 in QKV_ORDER:
                box = {}

                def mk(c, j01=j01, t4=t4, box=box):
                    def f():
                        if c == 0:
                            box["ps"] = lend.tile(
                                [P, QB], f32, name=f"q{hp}_{j01}_{t4}", tag="ln"
                            )
                        nc.tensor.matmul(
                            box["ps"][:, :],
                            wq[:, c, j01 * P:(j01 + 1) * P],
                            xt[:, c, t4 * QB:(t4 + 1) * QB],
                            start=(c == 0),
                            stop=(c == CCH - 1),
                        )
                    return f

                def fin(j01=j01, t4=t4, box=box):
                    bcol = hp if j01 == 0 else CCH // 2 + hp
                    nc.vector.tensor_scalar_add(
                        qksb[:, hp, j01, t4 * QB:(t4 + 1) * QB],
                        box["ps"][:],
                        bqksb[:, bcol:bcol + 1],
                    )

                for c in range(CCH):
                    quanta.append(mk(c))
                quanta.append(fin)
            return quanta

        def qkv(hp, wq):
            for q in qkv_quanta(hp, wq):
                q()

        def vchunk(t):
            # V for all 8 heads, token chunk t: x^T stationary, Wv moving.
            vps = lend.tile([P, CL], f32, name=f"vps{t}", tag="ln")
            for c in range(CCH):
                nc.tensor.matmul(
                    vps[:, :],
                    xt[:, c, t * P:(t + 1) * P],
                    wvsb[:, c, :],
                    start=(c == 0),
                    stop=False,
                )
            nc.tensor.matmul(vps[:, :], ones[:, :], bvsb[:, :], start=False, stop=True)
            nc.vector.tensor_copy(
                vsb.rearrange("p t (h c) -> p t h c", c=VW)[:, t, :, :D],
                vps[:, :].rearrange("p (h c) -> p h c", c=D),
            )

        def load_mask(mq, qb, mi):
            nc.sync.dma_start(
                mq[:, mi, :],
                maskk[mi * P:(mi + 1) * P, qb * QB:(qb + 1) * QB],
            )

        def attention_unit(hp, qb, mq, mq_next, next_qb, slotted, defer,
                           filler=None, carry_out=0):
            qlo = qb * QB
            otp = None
            prev = []  # deferred (mi, pt) list

            def emit_pv(pmi, ppt):
                for e in range(2):
                    h = 2 * hp + e
                    nc.tensor.matmul(
                        otp[e][:, :],
                        vsb[:, pmi, VW * h:VW * h + VW],
                        ppt[:, e, :],
                        start=(pmi == 0),
                        stop=(pmi == NCH - 1),
                    )

            for mi in range(NCH):
                st = stp.tile([P, 2, QB], f32, name=f"s{hp}_{qb}_{mi}", tag="st")
                for e in range(2):
                    row = D * e
                    nc.tensor.matmul(
                        st[:, e, :],
                        qksb[row:row + D, hp, 1, mi * P:(mi + 1) * P],
                        qksb[row:row + D, hp, 0, qlo:qlo + QB],
                        start=True,
                        stop=True,
                    )
                for th in slotted.get(mi, []):
                    th()
                if mi == 0:
                    otp = [
                        otp_p.tile([VW, QB], f32, name=f"ot{hp}_{qb}_{e}", tag="ot")
                        for e in range(2)
                    ]
                pt = ppool.tile([P, 2, QB], bf16, name=f"p{hp}_{qb}_{mi}", tag="pp")
                nc.scalar.activation(pt[:, :, :], st[:, :, :], Exp)
                nc.vector.tensor_tensor(
                    pt[:, :, :],
                    pt[:, :, :],
                    mq[:, mi, None, :].to_broadcast([P, 2, QB]),
                    mybir.AluOpType.mult,
                )
                if mq_next is not None:
                    load_mask(mq_next, next_qb, mi)
                if filler is not None:
                    filler(mi)
                prev.append((mi, pt))
                while len(prev) > defer + carry_out:
                    pmi, ppt = prev.pop(0)
                    emit_pv(pmi, ppt)
            while len(prev) > carry_out:
                pmi, ppt = prev.pop(0)
                emit_pv(pmi, ppt)
            carry = [
                (lambda pmi=pmi, ppt=ppt: emit_pv(pmi, ppt)) for pmi, ppt in prev
            ]

            def normalize_e(e):
                # normalize head e straight out of the PSUM accumulator:
                # row D of otp is the softmax denominator
                row = D * e
                ssb = spool.tile([1, QB], bf16, name=f"sb{hp}_{qb}_{e}", tag="sp")
                nc.vector.tensor_copy(ssb[:], otp[e][D:D + 1, :])
                sbc = lend.tile([D, QB], f32, name=f"sc{hp}_{qb}_{e}", tag="ln")
                nc.tensor.matmul(
                    sbc[:, :], ones[:, :D], ssb[:, :], start=True, stop=True
                )
                rb = spool.tile([D, QB], f32, name=f"rb{hp}_{qb}_{e}", tag="sp")
                nc.vector.reciprocal_approx_fast(rb[:], sbc[:])
                nc.vector.tensor_mul(
                    onrm[row:row + D, hp, qlo:qlo + QB],
                    otp[e][:D, :],
                    rb[:],
                )

            return [lambda: normalize_e(0), lambda: normalize_e(1)], carry

        def proj_quanta(tlist):
            quanta = []
            for t in tlist:
                for s in range(2):
                    box = {}

                    def mk(c, t=t, s=s, box=box):
                        def f():
                            if c == 0:
                                box["zp"] = lend.tile(
                                    [P, QB], f32, name=f"z{t}_{s}", tag="ln"
                                )
                            nc.tensor.matmul(
                                box["zp"][:, :],
                                onrm[:, c, t * P:(t + 1) * P],
                                wpsb[:, c, s * QB:(s + 1) * QB],
                                start=(c == 0),
                                stop=False,
                            )
                        return f

                    def fin(t=t, s=s, box=box):
                        nc.tensor.matmul(
                            box["zp"][:, :], ones[:, :], bpsb[:, s * QB:(s + 1) * QB],
                            start=False, stop=True,
                        )
                        zs = zpool.tile([P, QB], f32, name=f"zs{t}_{s}", tag="zs")
                        nc.vector.tensor_copy(zs[:], box["zp"][:])
                        nc.sync.dma_start(
                            out[t * P:(t + 1) * P, s * QB:(s + 1) * QB], zs[:]
                        )

                    for c in range(CL // P):
                        quanta.append(mk(c))
                    quanta.append(fin)
            return quanta

        # ---------------- emission schedule ----------------
        wq0 = load_wq(0)
        wq1 = load_wq(1)
        mq0 = mqpool.tile([P, NCH, QB], bf16, name="mq_0", tag="mq")
        for mi in range(NCH):
            load_mask(mq0, 0, mi)
        qkv(0, wq0)
        for c in range(CCH):
            nc.sync.dma_start(wvsb[:, c, :], wv[c * P:(c + 1) * P, :])

        units = [(hp, qb) for hp in range(HP) for qb in range(NQB)]
        pend = None
        carry = []
        wq_tiles = {0: wq0, 1: wq1}
        mq = mq0
        backlog = []  # pending filler quanta, drained a few per mi
        for ui, (hp, qb) in enumerate(units):
            defer = 2
            carry_out = 0
            qpm = 3 if (hp == HP - 1 or qb == 2) else 2
            if hp == 0 and qb == 0:
                # First unit: V chunks 0..9 interleave per-mi (vchunk(t)
                # precedes PV(mi=t)); the last 6 PVs are carried into the
                # next unit so its exp stream is not blocked by the V drain.
                defer, carry_out = 6, 6
                backlog = [(lambda t=t: vchunk(t)) for t in range(10)]
                qpm = 1
            elif qb == 1 and hp + 1 < HP:
                if hp == 0:
                    backlog.extend(lambda t=t: vchunk(t) for t in range(10, NCH))
                backlog.extend(qkv_quanta(hp + 1, wq_tiles[hp + 1]))
            if hp == HP - 1 and qb > 0:
                # norm(3, qb-1) was emitted via the pending slots in this
                # unit -> its q-range can be projected now.
                backlog.extend(proj_quanta(
                    range((qb - 1) * (NCH // NQB), qb * (NCH // NQB))
                ))

            def filler(mi, qpm=qpm):
                for _ in range(qpm):
                    if backlog:
                        backlog.pop(0)()

            # slot the previous unit's carried PVs, then its norms; with no
            # carry the norms land at mi 0 and 1 as before
            slotted = {}
            if carry:
                for k, th in enumerate(carry):
                    slotted.setdefault(1 + k // 2, []).append(th)
                base = 1 + (len(carry) + 1) // 2
                defer = max(defer, base + 2 + 1)
            else:
                base = 0
            if pend is not None:
                for j, nt in enumerate(pend):
                    slotted.setdefault(base + j, []).append(nt)

            if ui + 1 < len(units):
                nqb = units[ui + 1][1]
                mq_next = mqpool.tile([P, NCH, QB], bf16, name=f"mq_{ui+1}", tag="mq")
            else:
                nqb, mq_next = 0, None
            pend, carry = attention_unit(
                hp, qb, mq, mq_next, nqb, slotted, defer, filler, carry_out
            )
            mq = mq_next
            # stagger weight / proj-weight DMAs after units
            if qb == 2 and hp + 2 < HP:
                wq_tiles[hp + 2] = load_wq(hp + 2)
            elif hp == 1 and qb == 1:
                for o in range(CL // P):
                    nc.sync.dma_start(wpsb[:, o, :], wp[o * P:(o + 1) * P, :])
                nc.sync.dma_start(bpsb[:], bp[:])
        # drain whatever filler work remains, then the last q-block's proj
        for q in backlog:
            q()
        for p in pend:
            p()
        for q in proj_quanta(range((NQB - 1) * (NCH // NQB), NCH)):
            q()


def _build_nc():
    import concourse.tile as tile
    from concourse import bacc, mybir

    f32 = mybir.dt.float32
    bf16 = mybir.dt.bfloat16

    nc = bacc.Bacc("TRN2", target_bir_lowering=False, debug=False)

    xT = nc.declare_dram_parameter("xT", [DIM, N], bf16, isOutput=False)
    wqk = nc.declare_dram_parameter("wqk", [DIM, 2 * CL], bf16, isOutput=False)
    wv = nc.declare_dram_parameter("wv", [DIM, CL], bf16, isOutput=False)
    bqk = nc.declare_dram_parameter("bqk", [2 * CL], f32, isOutput=False)
    bv = nc.declare_dram_parameter("bv", [1, CL], bf16, isOutput=False)
    maskk = nc.declare_dram_parameter("maskk", [N, N], bf16, isOutput=False)
    wp = nc.declare_dram_parameter("wp", [CL, DIM], bf16, isOutput=False)
    bp = nc.declare_dram_parameter("bp", [1, DIM], bf16, isOutput=False)
    out = nc.declare_dram_parameter("out", [N, DIM], f32, isOutput=True)

    with tile.TileContext(nc) as tc:
        _body(tc, nc, mybir, xT, wqk, wv, bqk, bv, maskk, wp, bp, out)
    nc.compile()
    return nc


def _get_nc():
    global _nc_cache
    if _nc_cache is None:
        _nc_cache = _build_nc()
    return _nc_cache


def _shard_inputs(x, mask, Wqkv, bqkv, Wproj, bproj):
    x = np.asarray(x, np.float32)
    mask = np.asarray(mask)
    Wqkv = np.asarray(Wqkv, np.float32)
    bqkv = np.asarray(bqkv, np.float32)
    Wproj = np.asarray(Wproj, np.float32)
    bproj = np.asarray(bproj, np.float32)

    in_maps = []
    for core in range(8):
        b, g = divmod(core, 2)
        qs = slice(CL * g, CL * (g + 1))
        ks = slice(DIM + CL * g, DIM + CL * (g + 1))
        vs = slice(2 * DIM + CL * g, 2 * DIM + CL * (g + 1))
        # softmax 1/sqrt(D) folded into the K weights/bias
        wqk_np = np.concatenate([Wqkv[:, qs], Wqkv[:, ks] * 0.125], axis=1)
        bqk_np = np.concatenate([bqkv[qs], bqkv[ks] * 0.125])
        in_maps.append({
            "xT": np.ascontiguousarray(x[b].T).astype(BF16),
            "wqk": wqk_np.astype(BF16),
            "wv": np.ascontiguousarray(Wqkv[:, vs]).astype(BF16),
            "bqk": bqk_np.astype(np.float32),
            "bv": bqkv[vs].astype(BF16)[None, :],
            # [m, q] layout keep-mask; 1 = attend, 0 = masked (multiplied in)
            "maskk": np.ascontiguousarray(~mask[b].T).astype(BF16),
            "wp": np.ascontiguousarray(Wproj[CL * g:CL * (g + 1), :]).astype(BF16),
            "bp": (bproj if g == 0 else np.zeros_like(bproj)).astype(BF16)[None, :],
        })
    return in_maps


def _ensure_ntff_hook():
    """Inject an ``antenv.axon_hooks`` shim (absent on this image) and register
    the ctypes NTFF-profile hook against the loaded libaxon_pjrt.so, so
    ``run_bass_kernel_spmd(trace=True)`` can capture exec_time_ns."""
    import sys
    import types
    import ctypes
    import contextlib

    if "antenv.axon_hooks" not in sys.modules:
        mod = types.ModuleType("antenv.axon_hooks")
        mod._hook = None
        mod.set_axon_ntff_profile_hook = lambda h: setattr(mod, "_hook", h)
        mod.get_axon_ntff_profile_hook = lambda: mod._hook
        sys.modules["antenv.axon_hooks"] = mod
        import antenv

        antenv.axon_hooks = mod

    import antenv.axon_hooks as ah

    if ah.get_axon_ntff_profile_hook() is not None:
        return

    so_path = "/opt/axon/libaxon_pjrt.so"
    if not os.path.exists(so_path):
        return
    lib = ctypes.CDLL(so_path)
    if not hasattr(lib, "axon_start_nrt_profile"):
        return
    lib.axon_start_nrt_profile.argtypes = [
        ctypes.POINTER(ctypes.c_int64),
        ctypes.c_size_t,
    ]
    lib.axon_start_nrt_profile.restype = ctypes.c_int64
    lib.axon_stop_nrt_profile.argtypes = [ctypes.c_char_p]
    lib.axon_stop_nrt_profile.restype = ctypes.c_int64

    @contextlib.contextmanager
    def _hook(output_dir, device_ids):
        import jax

        jax.devices()
        if device_ids:
            ids = (ctypes.c_int64 * len(device_ids))(*device_ids)
            rc = lib.axon_start_nrt_profile(ids, len(device_ids))
        else:
            rc = lib.axon_start_nrt_profile(None, 0)
        if rc != 0:
            raise RuntimeError(f"axon_start_nrt_profile rc={rc}")
        try:
            yield
        finally:
            n = lib.axon_stop_nrt_profile(str(output_dir).encode())
            print(f"ntff profile: {n} file(s) written to {output_dir}")

    ah.set_axon_ntff_profile_hook(_hook)


def kernel(x, mask, Wqkv, bqkv, Wproj, bproj):
    global LAST_EXEC_NS, LAST_RESULTS
    from concourse.bass_utils import run_bass_kernel_spmd

    nc = _get_nc()
    in_maps = _shard_inputs(x, mask, Wqkv, bqkv, Wproj, bproj)
    profile = os.environ.get("BASS_ATTN_PROFILE", "0") == "1"
    if profile:
        _ensure_ntff_hook()
    res = run_bass_kernel_spmd(
        nc, in_maps, core_ids=list(range(8)), trace=profile
    )
    LAST_EXEC_NS = res.exec_time_ns
    LAST_RESULTS = res
    outs = [np.asarray(res.results[c]["out"], np.float32) for c in range(8)]
    return np.stack([outs[2 * b] + outs[2 * b + 1] for b in range(B)], axis=0)


# revision 62
# speedup vs baseline: 1.0125x; 1.0125x over previous
"""Distributed (8-NeuronCore SPMD) Trainium2 Bass kernel: masked multi-head attention.

Problem: x[4,2048,1024] -> qkv (16 heads, d=64) -> masked softmax attention -> proj.

Sharding (Megatron-style, per sharding hint):
  core c -> batch b = c//2, head-group g = c%2 (8 heads per core).
  Wqkv columns / Wproj rows are split by head group; each core computes a full
  [2048,1024] *partial* projection output for its batch; the host sums the two
  tensor-parallel partials per batch (bproj is fed to group 0 only).

Per-core layout (PSUM-bank-economy design):
  - Loop nest: head-pair hp OUTER (4), q-block qb INNER (4 x 512 tokens),
    m-chunk mi INNERMOST (16 x 128 tokens).
  - Scores S^T[m, q] per head pair land in one [128, 2, 512] f32 PSUM tile
    (2 banks); two such tiles double-buffer (4 banks).  The two heads' score
    matmuls go to disjoint PE row groups (partitions 0/64) and different
    banks, so they issue back-to-back and run concurrently on the array.
  - One merged ACT exp per mi covers both heads ([128, 1024] -> bf16 pt);
    ScalarE paces the loop at ~1.15us per m-chunk.
  - Keep-mask is bf16 (so the DVE multiply runs in 2x packed mode) and is
    streamed from HBM per unit ([128, 16, 512] tiles, double-buffered,
    prefetched during the previous unit; re-read once per head-pair pass).
  - PV: per head a [65, 512] f32 PSUM accumulator (1 bank each; V carries a
    ones column so row 64 accumulates the softmax denominator; normalization
    reads the accumulator directly from PSUM).
  - The remaining 2 PSUM banks host ALL filler matmuls as 1-bank [*, 512]
    tiles: qkv slices for hp+1, V chunks (inside the first unit), the
    denominator broadcast, and the output projection.  Filler work is split
    into single-matmul quanta drained a few per m-chunk, because Tile's
    cross-engine waits count the producer engine's stream position: anything
    scheduled ahead of a score matmul delays the exp that depends on it.
  - proj(qb) is emitted as soon as norm(hp=3, qb) is available, overlapping
    the last head-pair pass; output streamed per [128, 512] tile.
  - A dummy matmul burst at kernel start flips the PE HAM clock-gate to 8/8
    while x loads, so qkv(0) runs at 2.4 GHz.
"""

import os

import numpy as np
import ml_dtypes

BF16 = ml_dtypes.bfloat16

B, N, DIM, HEADS = 4, 2048, 1024, 16
HL = HEADS // 2        # heads per core = 8
D = DIM // HEADS       # head dim = 64
CL = DIM // 2          # local channels per core = 512
P = 128
NCH = N // P           # 16 token chunks
CCH = DIM // P         # 8 contraction chunks
VW = D + 1             # 65: V plus ones column
QB = 512               # q-block width
NQB = N // QB          # 4 q-blocks

_nc_cache = None
LAST_EXEC_NS = None
LAST_RESULTS = None


def _body(tc, nc, mybir, xT, wqk, wv, bqk, bv, maskk, wp, bp, out):
    import concourse.bass as bass  # noqa: F401

    f32 = mybir.dt.float32
    bf16 = mybir.dt.bfloat16
    Exp = mybir.ActivationFunctionType.Exp
    HP = HL // 2  # head pairs per core = 4

    with (
        tc.tile_pool(name="persist", bufs=1) as pers,
        tc.tile_pool(name="wqkp", bufs=2) as wqkpool,
        tc.tile_pool(name="pp", bufs=12) as ppool,     # pt (masked exp) tiles
        tc.tile_pool(name="mq", bufs=2) as mqpool,     # streamed bf16 keep-mask
        tc.tile_pool(name="op", bufs=4) as opool,      # ost / staging f32
        tc.tile_pool(name="sp", bufs=4) as spool,      # small norm tiles
        tc.tile_pool(name="zp", bufs=3) as zpool,      # proj staging
        tc.tile_pool(name="st", bufs=2, space="PSUM") as stp,    # 4 banks
        tc.tile_pool(name="ot", bufs=2, space="PSUM") as otp_p,  # 2 banks
        tc.tile_pool(name="ln", bufs=2, space="PSUM") as lend,   # 2 banks
    ):
        # ---------------- persistent tiles ----------------
        xt = pers.tile([P, CCH, N], bf16, name="xt")          # x^T, [c, n]
        vsb = pers.tile([P, NCH, HL * VW], bf16, name="vsb")  # V natural, [V | 1] per head
        wpsb = pers.tile([P, CL // P, DIM], bf16, name="wpsb")
        onrm = pers.tile([P, CL // P, N], bf16, name="onrm")  # normalized O^T, [c, n]
        qksb = pers.tile([P, HP, 2, N], bf16, name="qksb")    # Q^T,K^T all head pairs
        wvsb = pers.tile([P, CCH, CL], bf16, name="wvsb")
        bqksb = pers.tile([P, 2 * CL // P], f32, name="bqksb")
        bvsb = pers.tile([1, CL], bf16, name="bvsb")
        bpsb = pers.tile([1, DIM], bf16, name="bpsb")
        ones = pers.tile([1, P], bf16, name="ones")
        ones5 = pers.tile([1, QB], bf16, name="ones5")

        nc.vector.memset(ones[:], 1.0)
        nc.vector.memset(ones5[:], 1.0)
        nc.vector.memset(
            vsb.rearrange("p t (h c) -> p t h c", c=VW)[:, :, :, D], 1.0
        )
        # x first (everything depends on it), one big transfer per queue
        for c in range(CCH):
            nc.sync.dma_start(xt[:, c, :], xT[c * P:(c + 1) * P, :])
        # dense dummy matmul burst: flips the PE HAM clock-gate to 8/8
        # during the x DMA so qkv(0) runs at 2.4 GHz, not 1.2
        warm = lend.tile([P, QB], f32, name="warm", tag="ln")
        for i in range(20):
            nc.tensor.matmul(warm[:, :], ones[:, :], ones5[:, :],
                             start=(i == 0), stop=(i == 19))
        nc.sync.dma_start(bqksb[:], bqk.rearrange("(o p) -> p o", p=P))
        nc.sync.dma_start(bvsb[:], bv[:])

        def load_wq(hp):
            wq = wqkpool.tile([P, CCH, 2 * P], bf16, name=f"wq{hp}", tag="wqk")
            for c in range(CCH):
                nc.sync.dma_start(
                    wq[:, c, :P], wqk[c * P:(c + 1) * P, P * hp:P * (hp + 1)]
                )
                nc.sync.dma_start(
                    wq[:, c, P:], wqk[c * P:(c + 1) * P, CL + P * hp:CL + P * (hp + 1)]
                )
            return wq

        QKV_ORDER = [(0, 0)] + [(1, t4) for t4 in range(N // QB)] + [
            (0, t4) for t4 in range(1, N // QB)
        ]

        def qkv_quanta(hp, wq):
            # One thunk per matmul so filler work interleaves at ~215ns
            # granularity inside the ACT-paced loop; the lend slot for a
            # slice is held across its 8 accumulation quanta.
            quanta = []
            for j01, t4 in QKV_ORDER:
                box = {}

                def mk(c, j01=j01, t4=t4, box=box):
                    def f():
                        if c == 0:
                            box["ps"] = lend.tile(
                                [P, QB], f32, name=f"q{hp}_{j01}_{t4}", tag="ln"
                            )
                        nc.tensor.matmul(
                            box["ps"][:, :],
                            wq[:, c, j01 * P:(j01 + 1) * P],
                            xt[:, c, t4 * QB:(t4 + 1) * QB],
                            start=(c == 0),
                            stop=(c == CCH - 1),
                        )
                    return f

                def fin(j01=j01, t4=t4, box=box):
                    bcol = hp if j01 == 0 else CCH // 2 + hp
                    nc.vector.tensor_scalar_add(
                        qksb[:, hp, j01, t4 * QB:(t4 + 1) * QB],
                        box["ps"][:],
                        bqksb[:, bcol:bcol + 1],
                    )

                for c in range(CCH):
                    quanta.append(mk(c))
                quanta.append(fin)
            return quanta

        def qkv(hp, wq):
            for q in qkv_quanta(hp, wq):
                q()

        def vchunk(t):
            # V for all 8 heads, token chunk t: x^T stationary, Wv moving.
            vps = lend.tile([P, CL], f32, name=f"vps{t}", tag="ln")
            for c in range(CCH):
                nc.tensor.matmul(
                    vps[:, :],
                    xt[:, c, t * P:(t + 1) * P],
                    wvsb[:, c, :],
                    start=(c == 0),
                    stop=False,
                )
            nc.tensor.matmul(vps[:, :], ones[:, :], bvsb[:, :], start=False, stop=True)
            nc.vector.tensor_copy(
                vsb.rearrange("p t (h c) -> p t h c", c=VW)[:, t, :, :D],
                vps[:, :].rearrange("p (h c) -> p h c", c=D),
            )

        def load_mask(mq, qb, mi):
            nc.sync.dma_start(
                mq[:, mi, :],
                maskk[mi * P:(mi + 1) * P, qb * QB:(qb + 1) * QB],
            )

        def attention_unit(hp, qb, mq, mq_next, next_qb, slotted, defer,
                           filler=None, carry_out=0):
            qlo = qb * QB
            otp = None
            prev = []  # deferred (mi, pt) list

            def emit_pv(pmi, ppt):
                for e in range(2):
                    h = 2 * hp + e
                    nc.tensor.matmul(
                        otp[e][:, :],
                        vsb[:, pmi, VW * h:VW * h + VW],
                        ppt[:, e, :],
                        start=(pmi == 0),
                        stop=(pmi == NCH - 1),
                    )

            for mi in range(NCH):
                st = stp.tile([P, 2, QB], f32, name=f"s{hp}_{qb}_{mi}", tag="st")
                for e in range(2):
                    row = D * e
                    nc.tensor.matmul(
                        st[:, e, :],
                        qksb[row:row + D, hp, 1, mi * P:(mi + 1) * P],
                        qksb[row:row + D, hp, 0, qlo:qlo + QB],
                        start=True,
                        stop=True,
                    )
                for th in slotted.get(mi, []):
                    th()
                if mi == 0:
                    otp = [
                        otp_p.tile([VW, QB], f32, name=f"ot{hp}_{qb}_{e}", tag="ot")
                        for e in range(2)
                    ]
                pt = ppool.tile([P, 2, QB], bf16, name=f"p{hp}_{qb}_{mi}", tag="pp")
                nc.scalar.activation(pt[:, :, :], st[:, :, :], Exp)
                nc.vector.tensor_tensor(
                    pt[:, :, :],
                    pt[:, :, :],
                    mq[:, mi, None, :].to_broadcast([P, 2, QB]),
                    mybir.AluOpType.mult,
                )
                if mq_next is not None:
                    load_mask(mq_next, next_qb, mi)
                if filler is not None:
                    filler(mi)
                prev.append((mi, pt))
                while len(prev) > defer + carry_out:
                    pmi, ppt = prev.pop(0)
                    emit_pv(pmi, ppt)
            while len(prev) > carry_out:
                pmi, ppt = prev.pop(0)
                emit_pv(pmi, ppt)
            carry = [
                (lambda pmi=pmi, ppt=ppt: emit_pv(pmi, ppt)) for pmi, ppt in prev
            ]

            def normalize_e(e):
                # normalize head e straight out of the PSUM accumulator:
                # row D of otp is the softmax denominator
                row = D * e
                ssb = spool.tile([1, QB], bf16, name=f"sb{hp}_{qb}_{e}", tag="sp")
                nc.vector.tensor_copy(ssb[:], otp[e][D:D + 1, :])
                sbc = lend.tile([D, QB], f32, name=f"sc{hp}_{qb}_{e}", tag="ln")
                nc.tensor.matmul(
                    sbc[:, :], ones[:, :D], ssb[:, :], start=True, stop=True
                )
                rb = spool.tile([D, QB], f32, name=f"rb{hp}_{qb}_{e}", tag="sp")
                nc.vector.reciprocal_approx_fast(rb[:], sbc[:])
                nc.vector.tensor_mul(
                    onrm[row:row + D, hp, qlo:qlo + QB],
                    otp[e][:D, :],
                    rb[:],
                )

            return [lambda: normalize_e(0), lambda: normalize_e(1)], carry

        def proj_quanta(tlist):
            quanta = []
            for t in tlist:
                for s in range(2):
                    box = {}

                    def mk(c, t=t, s=s, box=box):
                        def f():
                            if c == 0:
                                box["zp"] = lend.tile(
                                    [P, QB], f32, name=f"z{t}_{s}", tag="ln"
                                )
                            nc.tensor.matmul(
                                box["zp"][:, :],
                                onrm[:, c, t * P:(t + 1) * P],
                                wpsb[:, c, s * QB:(s + 1) * QB],
                                start=(c == 0),
                                stop=False,
                            )
                        return f

                    def fin(t=t, s=s, box=box):
                        nc.tensor.matmul(
                            box["zp"][:, :], ones[:, :], bpsb[:, s * QB:(s + 1) * QB],
                            start=False, stop=True,
                        )
                        zs = zpool.tile([P, QB], f32, name=f"zs{t}_{s}", tag="zs")
                        nc.vector.tensor_copy(zs[:], box["zp"][:])
                        nc.sync.dma_start(
                            out[t * P:(t + 1) * P, s * QB:(s + 1) * QB], zs[:]
                        )

                    for c in range(CL // P):
                        quanta.append(mk(c))
                    quanta.append(fin)
            return quanta

        # ---------------- emission schedule ----------------
        wq0 = load_wq(0)
        wq1 = load_wq(1)
        mq0 = mqpool.tile([P, NCH, QB], bf16, name="mq_0", tag="mq")
        for mi in range(NCH):
            load_mask(mq0, 0, mi)
        qkv(0, wq0)
        for c in range(CCH):
            nc.sync.dma_start(wvsb[:, c, :], wv[c * P:(c + 1) * P, :])

        units = [(hp, qb) for hp in range(HP) for qb in range(NQB)]
        pend = None
        carry = []
        wq_tiles = {0: wq0, 1: wq1}
        mq = mq0
        backlog = []  # pending filler quanta, drained a few per mi
        for ui, (hp, qb) in enumerate(units):
            defer = 2
            carry_out = 0
            qpm = 3 if hp == HP - 1 else 2
            if hp == 0 and qb == 0:
                # First unit: V chunks 0..9 interleave per-mi (vchunk(t)
                # precedes PV(mi=t)); the last 6 PVs are carried into the
                # next unit so its exp stream is not blocked by the V drain.
                defer, carry_out = 6, 6
                backlog = [(lambda t=t: vchunk(t)) for t in range(10)]
                qpm = 1
            elif qb == 1 and hp + 1 < HP:
                if hp == 0:
                    backlog.extend(lambda t=t: vchunk(t) for t in range(10, NCH))
                backlog.extend(qkv_quanta(hp + 1, wq_tiles[hp + 1]))
            if hp == HP - 1 and qb > 0:
                # norm(3, qb-1) was emitted via the pending slots in this
                # unit -> its q-range can be projected now.
                backlog.extend(proj_quanta(
                    range((qb - 1) * (NCH // NQB), qb * (NCH // NQB))
                ))

            def filler(mi, qpm=qpm):
                for _ in range(qpm):
                    if backlog:
                        backlog.pop(0)()

            # slot the previous unit's carried PVs, then its norms; with no
            # carry the norms land at mi 0 and 1 as before
            slotted = {}
            if carry:
                for k, th in enumerate(carry):
                    slotted.setdefault(1 + k // 2, []).append(th)
                base = 1 + (len(carry) + 1) // 2
                defer = max(defer, base + 3)
            else:
                base = 0
            if pend is not None:
                for j, nt in enumerate(pend):
                    slotted.setdefault(base + j, []).append(nt)

            if ui + 1 < len(units):
                nqb = units[ui + 1][1]
                mq_next = mqpool.tile([P, NCH, QB], bf16, name=f"mq_{ui+1}", tag="mq")
            else:
                nqb, mq_next = 0, None
            pend, carry = attention_unit(
                hp, qb, mq, mq_next, nqb, slotted, defer, filler, carry_out
            )
            mq = mq_next
            # stagger weight / proj-weight DMAs after units
            if qb == 2 and hp + 2 < HP:
                wq_tiles[hp + 2] = load_wq(hp + 2)
            elif hp == 1 and qb == 1:
                for o in range(CL // P):
                    nc.sync.dma_start(wpsb[:, o, :], wp[o * P:(o + 1) * P, :])
                nc.sync.dma_start(bpsb[:], bp[:])
        # drain whatever filler work remains, then the last q-block's proj
        for q in backlog:
            q()
        for p in pend:
            p()
        for q in proj_quanta(range((NQB - 1) * (NCH // NQB), NCH)):
            q()


def _build_nc():
    import concourse.tile as tile
    from concourse import bacc, mybir

    f32 = mybir.dt.float32
    bf16 = mybir.dt.bfloat16

    nc = bacc.Bacc("TRN2", target_bir_lowering=False, debug=False)

    xT = nc.declare_dram_parameter("xT", [DIM, N], bf16, isOutput=False)
    wqk = nc.declare_dram_parameter("wqk", [DIM, 2 * CL], bf16, isOutput=False)
    wv = nc.declare_dram_parameter("wv", [DIM, CL], bf16, isOutput=False)
    bqk = nc.declare_dram_parameter("bqk", [2 * CL], f32, isOutput=False)
    bv = nc.declare_dram_parameter("bv", [1, CL], bf16, isOutput=False)
    maskk = nc.declare_dram_parameter("maskk", [N, N], bf16, isOutput=False)
    wp = nc.declare_dram_parameter("wp", [CL, DIM], bf16, isOutput=False)
    bp = nc.declare_dram_parameter("bp", [1, DIM], bf16, isOutput=False)
    out = nc.declare_dram_parameter("out", [N, DIM], f32, isOutput=True)

    with tile.TileContext(nc) as tc:
        _body(tc, nc, mybir, xT, wqk, wv, bqk, bv, maskk, wp, bp, out)
    nc.compile()
    return nc


def _get_nc():
    global _nc_cache
    if _nc_cache is None:
        _nc_cache = _build_nc()
    return _nc_cache


def _shard_inputs(x, mask, Wqkv, bqkv, Wproj, bproj):
    x = np.asarray(x, np.float32)
    mask = np.asarray(mask)
    Wqkv = np.asarray(Wqkv, np.float32)
    bqkv = np.asarray(bqkv, np.float32)
    Wproj = np.asarray(Wproj, np.float32)
    bproj = np.asarray(bproj, np.float32)

    in_maps = []
    for core in range(8):
        b, g = divmod(core, 2)
        qs = slice(CL * g, CL * (g + 1))
        ks = slice(DIM + CL * g, DIM + CL * (g + 1))
        vs = slice(2 * DIM + CL * g, 2 * DIM + CL * (g + 1))
        # softmax 1/sqrt(D) folded into the K weights/bias
        wqk_np = np.concatenate([Wqkv[:, qs], Wqkv[:, ks] * 0.125], axis=1)
        bqk_np = np.concatenate([bqkv[qs], bqkv[ks] * 0.125])
        in_maps.append({
            "xT": np.ascontiguousarray(x[b].T).astype(BF16),
            "wqk": wqk_np.astype(BF16),
            "wv": np.ascontiguousarray(Wqkv[:, vs]).astype(BF16),
            "bqk": bqk_np.astype(np.float32),
            "bv": bqkv[vs].astype(BF16)[None, :],
            # [m, q] layout keep-mask; 1 = attend, 0 = masked (multiplied in)
            "maskk": np.ascontiguousarray(~mask[b].T).astype(BF16),
            "wp": np.ascontiguousarray(Wproj[CL * g:CL * (g + 1), :]).astype(BF16),
            "bp": (bproj if g == 0 else np.zeros_like(bproj)).astype(BF16)[None, :],
        })
    return in_maps


def _ensure_ntff_hook():
    """Inject an ``antenv.axon_hooks`` shim (absent on this image) and register
    the ctypes NTFF-profile hook against the loaded libaxon_pjrt.so, so
    ``run_bass_kernel_spmd(trace=True)`` can capture exec_time_ns."""
    import sys
    import types
    import ctypes
    import contextlib

    if "antenv.axon_hooks" not in sys.modules:
        mod = types.ModuleType("antenv.axon_hooks")
        mod._hook = None
        mod.set_axon_ntff_profile_hook = lambda h: setattr(mod, "_hook", h)
        mod.get_axon_ntff_profile_hook = lambda: mod._hook
        sys.modules["antenv.axon_hooks"] = mod
        import antenv

        antenv.axon_hooks = mod

    import antenv.axon_hooks as ah

    if ah.get_axon_ntff_profile_hook() is not None:
        return

    so_path = "/opt/axon/libaxon_pjrt.so"
    if not os.path.exists(so_path):
        return
    lib = ctypes.CDLL(so_path)
    if not hasattr(lib, "axon_start_nrt_profile"):
        return
    lib.axon_start_nrt_profile.argtypes = [
        ctypes.POINTER(ctypes.c_int64),
        ctypes.c_size_t,
    ]
    lib.axon_start_nrt_profile.restype = ctypes.c_int64
    lib.axon_stop_nrt_profile.argtypes = [ctypes.c_char_p]
    lib.axon_stop_nrt_profile.restype = ctypes.c_int64

    @contextlib.contextmanager
    def _hook(output_dir, device_ids):
        import jax

        jax.devices()
        if device_ids:
            ids = (ctypes.c_int64 * len(device_ids))(*device_ids)
            rc = lib.axon_start_nrt_profile(ids, len(device_ids))
        else:
            rc = lib.axon_start_nrt_profile(None, 0)
        if rc != 0:
            raise RuntimeError(f"axon_start_nrt_profile rc={rc}")
        try:
            yield
        finally:
            n = lib.axon_stop_nrt_profile(str(output_dir).encode())
            print(f"ntff profile: {n} file(s) written to {output_dir}")

    ah.set_axon_ntff_profile_hook(_hook)


def kernel(x, mask, Wqkv, bqkv, Wproj, bproj):
    global LAST_EXEC_NS, LAST_RESULTS
    from concourse.bass_utils import run_bass_kernel_spmd

    nc = _get_nc()
    in_maps = _shard_inputs(x, mask, Wqkv, bqkv, Wproj, bproj)
    profile = os.environ.get("BASS_ATTN_PROFILE", "0") == "1"
    if profile:
        _ensure_ntff_hook()
    res = run_bass_kernel_spmd(
        nc, in_maps, core_ids=list(range(8)), trace=profile
    )
    LAST_EXEC_NS = res.exec_time_ns
    LAST_RESULTS = res
    outs = [np.asarray(res.results[c]["out"], np.float32) for c in range(8)]
    return np.stack([outs[2 * b] + outs[2 * b + 1] for b in range(B)], axis=0)
